# revision 10
# baseline (speedup 1.0000x reference)
"""Trainium2 Bass kernel for CLDOdeBlock (graph ODE, RK4 over batch-sharded cores).

Math (per batch b):
    An = adjacency / max(adjacency.sum(-1, keepdims=True), 1)
    vector_field(t, h) = tanh([h | An@h | te(t)] @ W1 + b1) @ W2 + b2
    RK4 with 8 steps over time_grid; output trajectory [B, T, C, D].

Restructured formulation (v-path): using associativity
    (An@h) @ W1a == An @ (h @ W1a)
the whole evaluation runs from the TRANSPOSED state hT [d, i] only — no PE
transposes, no transpose evacuations:
    v    = h @ W1a                      (natural [j, f], from hT chunks)
    out1T = W1h.T @ hT + sum_j v.T @ AnT   (PSUM accumulate, [f, i])
    a1T  = tanh(out1T + b1_eff)         (ACT, bias per-partition since f on parts)
    out2T = W2.T @ a1T                  ([d, i]); k = out2T + b2T
State updates (hs, h_new) stay in the T layout; b2 enters via per-partition
ACT bias (kw evac) and a per-step hb = h + (dt/2)*b2T precompute.
The Fourier time-embedding columns of W1 are folded into a per-eval bias
b1_eff = b1 + te(t) @ W1[512:544] on the host.

Engine balance per eval: PE 28672 rows (the only transposes-free floor),
ACT: tanh + weighted-k evac (bias fused), DVE: v evac + hs/hnew stt + hb,
Pool(gpsimd): RK4 accumulator adds. PSUM cycles through 4 two-bank slots.

float32r (TF32) is used for all matmul inputs: host inputs pre-rounded to
TF32; on-device producers of matmul operands (DVE/ACT evacs, state stt)
write through f32r-typed output APs.
"""

import math
from contextlib import ExitStack, nullcontext

import numpy as np

import concourse.bass as bass
import concourse.tile as tile
from concourse import bacc, mybir
from concourse.bass import ds

B, C, D = 16, 1024, 256
T = 9
NSTEP_FULL = T - 1
NCORES = 8
BPC = B // NCORES  # batches per core
TIME_DIM = 32
HALF = TIME_DIM // 2
F32 = mybir.dt.float32
F32R = mybir.dt.float32r

RT = C // 128   # 8 row (node) tiles
DT = D // 128   # 2 feature tiles
NH = C // 512   # 2 free halves for N=512 matmuls


def build_program(dts, n_steps=NSTEP_FULL, n_iters=1, use_f32r=True,
                  skip_stores=False, acc_on_pool=False):
    """Build + compile the per-core Bass program.

    dts: python floats, len n_steps (the RK4 dt per step; baked in).
    n_iters: >1 wraps the whole computation in a For_i loop (for timing).
    """
    nc = bacc.Bacc("TRN2", target_bir_lowering=False, debug=False)

    at_d = nc.dram_tensor("at", [BPC, RT, 128, C], F32, kind="ExternalInput").ap()
    h0_d = nc.dram_tensor("h0", [BPC, 128, DT, C], F32, kind="ExternalInput").ap()
    w1h_d = nc.dram_tensor("w1h", [128, DT, D], F32, kind="ExternalInput").ap()
    w1a_d = nc.dram_tensor("w1a", [128, DT, D], F32, kind="ExternalInput").ap()
    w2_d = nc.dram_tensor("w2n", [128, DT, D], F32, kind="ExternalInput").ap()
    b1_d = nc.dram_tensor("b1t", [128, DT, 4 * NSTEP_FULL], F32, kind="ExternalInput").ap()
    b2_d = nc.dram_tensor("b2t", [128, DT, 2], F32, kind="ExternalInput").ap()
    tr_d = nc.dram_tensor("traj", [BPC, n_steps, 128, DT, C], F32, kind="ExternalOutput").ap()

    def mm(ap):
        return ap.bitcast(F32R) if use_f32r else ap

    acc_eng = None  # set after engines exist

    with ExitStack() as ctx:
        tc = ctx.enter_context(tile.TileContext(nc))
        const = ctx.enter_context(tc.tile_pool(name="const", bufs=1))
        at_p = ctx.enter_context(tc.tile_pool(name="atp", bufs=1))

        # ---- constants / weights ----
        at_sb = at_p.tile([128, BPC, RT, C], F32)
        for b in range(BPC):
            for jc in range(RT):
                nc.sync.dma_start(mm(at_sb[:, b, jc, :]), mm(at_d[b, jc]))
        w1h_sb = const.tile([128, DT, D], F32)
        nc.sync.dma_start(mm(w1h_sb[:]), mm(w1h_d))
        w1a_sb = const.tile([128, DT, D], F32)
        nc.sync.dma_start(mm(w1a_sb[:]), mm(w1a_d))
        w2_sb = const.tile([128, DT, D], F32)
        nc.sync.dma_start(mm(w2_sb[:]), mm(w2_d))
        b1_sb = const.tile([128, DT, 4 * NSTEP_FULL], F32)
        nc.sync.dma_start(b1_sb[:], b1_d)
        b2_sb = const.tile([128, DT, 2], F32)  # [:, :, 0] = b2T, [:, :, 1] = 2*b2T
        nc.sync.dma_start(b2_sb[:], b2_d)

        # b2T broadcast along i: [128, DT, C] view of b2_sb[:, :, 0]
        b2_ap = b2_sb[:, :, 0]
        b2_bc = bass.AP(
            tensor=b2_ap.tensor,
            offset=b2_ap.offset,
            ap=[b2_ap.ap[0], b2_ap.ap[1], [0, C]],
        )

        # ---- main pools ----
        state_p = ctx.enter_context(tc.tile_pool(name="state", bufs=3))
        hs_p = ctx.enter_context(tc.tile_pool(name="hs", bufs=2))
        hb_p = ctx.enter_context(tc.tile_pool(name="hb", bufs=2))
        v_p = ctx.enter_context(tc.tile_pool(name="v", bufs=2))
        a1_p = ctx.enter_context(tc.tile_pool(name="a1", bufs=2))
        kw_p = ctx.enter_context(tc.tile_pool(name="kw", bufs=4))
        acc_p = ctx.enter_context(tc.tile_pool(name="acc", bufs=2))
        ps_p = ctx.enter_context(tc.tile_pool(name="ps", bufs=4, space="PSUM"))

        acc_eng = nc.gpsimd if acc_on_pool else nc.vector

        loop_cm = tc.For_i(0, n_iters) if n_iters > 1 else nullcontext()
        with loop_cm:
            hstates = []
            for b in range(BPC):
                hst = state_p.tile([128, DT, C], F32, tag="hst")
                nc.sync.dma_start(mm(hst[:]), mm(h0_d[b]))
                hstates.append(hst)

            for s in range(n_steps):
                dt = float(dts[s])
                hb_half = [None] * BPC  # h + (dt/2) * b2T
                accs = [None] * BPC     # running k1 + 2k2 + 2k3 + k4 (in kw1's tile)
                hstage = [None] * BPC
                for b in range(BPC):
                    hb = hb_p.tile([128, DT, C], F32, tag="hb")
                    nc.vector.scalar_tensor_tensor(
                        hb[:], b2_bc, dt / 2.0, hstates[b][:],
                        mybir.AluOpType.mult, mybir.AluOpType.add,
                    )
                    hb_half[b] = hb

                for g in range(4):
                    ev = s * 4 + g
                    w_g = 2.0 if g in (1, 2) else 1.0

                    h_in = [hstates[b] if g == 0 else hstage[b] for b in range(BPC)]
                    vsb = [None] * BPC
                    a1s = [None] * BPC
                    p2s = [[None] * DT for _ in range(BPC)]
                    kws = [None] * BPC

                    def phase_A(b):
                        # v = h @ W1a (natural [j, f]); evac on ACT (f32r)
                        vs = v_p.tile([128, RT, D], F32, tag="v", name="vs")
                        vsb[b] = vs
                        for jq in range(2):
                            pv = ps_p.tile([128, 4, D], F32, tag="ps", name="pv")
                            # region-outer, contraction-inner: a later region's
                            # `start` pending-zeroes the whole bank and must
                            # not hit an earlier region mid-accumulation
                            for jj in range(4):
                                jc = jq * 4 + jj
                                for d_ in range(DT):
                                    nc.tensor.matmul(
                                        pv[:, jj, :],
                                        mm(h_in[b][:, d_, ds(jc * 128, 128)]),
                                        mm(w1a_sb[:, d_, :]),
                                        start=(d_ == 0),
                                        stop=(d_ == DT - 1),
                                    )
                            nc.scalar.copy(mm(vs[:, ds(jq * 4, 4), :]), pv[:])

                    def phase_B(b):
                        # out1T = W1h.T @ hT + sum_jc v.T @ AnT; tanh evac
                        a1 = a1_p.tile([128, DT, C], F32, tag="a1", name="a1")
                        a1s[b] = a1
                        for f_ in range(DT):
                            p1 = ps_p.tile([128, C], F32, tag="ps", name="p1")
                            for nh in range(NH):
                                for d_ in range(DT):
                                    nc.tensor.matmul(
                                        p1[:, ds(nh * 512, 512)],
                                        mm(w1h_sb[:, d_, ds(f_ * 128, 128)]),
                                        mm(h_in[b][:, d_, ds(nh * 512, 512)]),
                                        start=(d_ == 0),
                                        stop=False,
                                    )
                            for nh in range(NH):
                                for jc in range(RT):
                                    nc.tensor.matmul(
                                        p1[:, ds(nh * 512, 512)],
                                        mm(vsb[b][:, jc, ds(f_ * 128, 128)]),
                                        mm(at_sb[:, b, jc, ds(nh * 512, 512)]),
                                        start=False,
                                        stop=(jc == RT - 1),
                                    )
                            nc.scalar.activation(
                                mm(a1[:, f_, :]),
                                p1[:],
                                mybir.ActivationFunctionType.Tanh,
                                bias=b1_sb[:, f_, ev : ev + 1],
                                scale=1.0,
                            )

                    def phase_C(b):
                        # out2T = W2.T @ a1T; kw = w_g*(out2T + b2T)
                        kw = kw_p.tile([128, DT, C], F32, tag="kw", name="kw")
                        kws[b] = kw
                        for m_ in range(DT):
                            p2 = ps_p.tile([128, C], F32, tag="ps", name="p2")
                            p2s[b][m_] = p2
                            for nh in range(NH):
                                for f_ in range(DT):
                                    nc.tensor.matmul(
                                        p2[:, ds(nh * 512, 512)],
                                        mm(w2_sb[:, f_, ds(m_ * 128, 128)]),
                                        mm(a1s[b][:, f_, ds(nh * 512, 512)]),
                                        start=(f_ == 0),
                                        stop=(f_ == DT - 1),
                                    )
                            bi = 1 if w_g == 2.0 else 0
                            nc.scalar.activation(
                                kw[:, m_, :],
                                p2[:],
                                mybir.ActivationFunctionType.Identity,
                                bias=b2_sb[:, m_, bi : bi + 1],
                                scale=w_g,
                            )

                    def update(b):
                        # RK4 state update for batch b (hs first: it gates the
                        # next stage's matmuls; acc adds drain later)
                        if g < 3:
                            hs = hs_p.tile([128, DT, C], F32, tag="hs", name="hs")
                            if g < 2:
                                # c = dt/2: hs = hb_half + (dt/2)*out2T (PSUM)
                                for d_ in range(DT):
                                    nc.vector.scalar_tensor_tensor(
                                        mm(hs[:, d_, :]), p2s[b][d_][:], dt / 2.0,
                                        hb_half[b][:, d_, :],
                                        mybir.AluOpType.mult, mybir.AluOpType.add,
                                    )
                            else:
                                # c = dt: hs = h + (dt/2)*kw3  (kw3 = 2*k3)
                                for d_ in range(DT):
                                    nc.vector.scalar_tensor_tensor(
                                        mm(hs[:, d_, :]), kws[b][:, d_, :], dt / 2.0,
                                        hstates[b][:, d_, :],
                                        mybir.AluOpType.mult, mybir.AluOpType.add,
                                    )
                            hstage[b] = hs
                        # accumulator: acc = kw1 + kw2 + kw3 + kw4
                        if g == 0:
                            accs[b] = kws[b]  # remember kw1; no op yet
                        elif g == 1:
                            a = acc_p.tile([128, DT, C], F32, tag="acc", name="a")
                            acc_eng.tensor_add(a[:], kws[b][:], accs[b][:])
                            accs[b] = a
                        else:
                            acc_eng.tensor_add(accs[b][:], kws[b][:], accs[b][:])
                        if g == 3:
                            hn = state_p.tile([128, DT, C], F32, tag="hst", name="hn")
                            for d_ in range(DT):
                                nc.vector.scalar_tensor_tensor(
                                    mm(hn[:, d_, :]), accs[b][:, d_, :], dt / 6.0,
                                    hstates[b][:, d_, :],
                                    mybir.AluOpType.mult, mybir.AluOpType.add,
                                )
                            hstates[b] = hn
                            if not skip_stores:
                                nc.sync.dma_start(tr_d[b, s], hn[:])

                    # software-pipelined issue order: each batch's evac/update
                    # tail hides under the other batch's PE phases
                    phase_A(0)
                    phase_B(0)
                    phase_A(1)
                    phase_C(0)
                    update(0)
                    phase_B(1)
                    phase_C(1)
                    update(1)

    nc.compile()
    return nc


def tf32_round(x):
    """Round-to-nearest-even to TF32 (10 mantissa bits) — what the PE's
    FP32r mode expects its operands to already be."""
    u = np.ascontiguousarray(x, np.float32).view(np.uint32)
    lsb = (u >> np.uint32(13)) & np.uint32(1)
    u = u + np.uint32(0x0FFF) + lsb
    u &= np.uint32(0xFFFFE000)
    return u.view(np.float32)


def host_prep(h0, time_grid, adjacency, W1, b1, W2, b2, n_steps=NSTEP_FULL,
              use_f32r=True):
    """Returns (in_maps list per core, dts list)."""
    h0 = np.asarray(h0, np.float32)
    time_grid = np.asarray(time_grid, np.float32)
    adjacency = np.asarray(adjacency, np.float32)
    W1 = np.asarray(W1, np.float32)
    b1 = np.asarray(b1, np.float32)
    W2 = np.asarray(W2, np.float32)
    b2 = np.asarray(b2, np.float32)

    # degree normalization (time-constant input prep)
    deg = np.maximum(adjacency.sum(-1, keepdims=True), np.float32(1.0))
    adjacency = (adjacency / deg).astype(np.float32)

    rnd = tf32_round if use_f32r else (lambda x: x)

    # Fourier features folded into per-eval bias
    freqs = np.exp(
        -math.log(10000.0) * np.arange(HALF, dtype=np.float32) / np.float32(HALF)
    ).astype(np.float32)

    def te(t):
        a = (np.float32(t) * freqs).astype(np.float32)
        return np.concatenate([np.sin(a), np.cos(a)]).astype(np.float32)

    dts = []
    b1_eff = np.zeros((4 * NSTEP_FULL, D), np.float32)
    for s in range(NSTEP_FULL):
        t0 = np.float32(time_grid[s])
        t1 = np.float32(time_grid[s + 1])
        dt = np.float32(t1 - t0)
        dts.append(float(dt))
        stage_ts = [t0, np.float32(t0 + dt / 2), np.float32(t0 + dt / 2),
                    np.float32(t0 + dt)]
        for g, tg_ in enumerate(stage_ts):
            b1_eff[s * 4 + g] = b1 + te(tg_) @ W1[2 * D:]

    # W1h/W1a in [d%128, d//128, f] layout; W2 in [f%128, f//128, d]
    w1h_in = rnd(np.ascontiguousarray(W1[:D].reshape(DT, 128, D).transpose(1, 0, 2)))
    w1a_in = rnd(np.ascontiguousarray(W1[D:2 * D].reshape(DT, 128, D).transpose(1, 0, 2)))
    w2_in = rnd(np.ascontiguousarray(W2.reshape(DT, 128, D).transpose(1, 0, 2)))
    b1_in = np.ascontiguousarray(
        b1_eff.reshape(4 * NSTEP_FULL, DT, 128).transpose(2, 1, 0)
    )
    # b2T in [d%128, d//128, {1x, 2x}]
    b2t = b2.reshape(DT, 128).transpose(1, 0)
    b2_in = np.ascontiguousarray(np.stack([b2t, 2.0 * b2t], axis=-1))

    in_maps = []
    for ci in range(NCORES):
        sl = slice(ci * BPC, (ci + 1) * BPC)
        at_in = rnd(np.ascontiguousarray(
            adjacency[sl].transpose(0, 2, 1).reshape(BPC, RT, 128, C)
        ))
        # h0T: [BPC, d%128, d//128, i]
        h0_in = rnd(np.ascontiguousarray(
            h0[sl].reshape(BPC, C, DT, 128).transpose(0, 3, 2, 1)))
        in_maps.append(
            {
                "at": at_in,
                "h0": h0_in,
                "w1h": w1h_in,
                "w1a": w1a_in,
                "w2n": w2_in,
                "b1t": b1_in,
                "b2t": b2_in,
            }
        )
    return in_maps, dts[:n_steps]


def gather(results, h0, n_steps=NSTEP_FULL):
    h0 = np.asarray(h0, np.float32)
    out = np.empty((B, n_steps + 1, C, D), np.float32)
    out[:, 0] = h0
    for ci in range(NCORES):
        t = results[ci]["traj"]  # [BPC, n_steps, 128, DT, C]
        out[ci * BPC : (ci + 1) * BPC, 1:] = t.transpose(0, 1, 4, 3, 2).reshape(
            BPC, n_steps, C, D
        )
    return out


_CACHE = {}


def kernel(h0, time_grid, adjacency, W1, b1, W2, b2):
    from concourse.bass_utils import run_bass_kernel_spmd

    in_maps, dts = host_prep(h0, time_grid, adjacency, W1, b1, W2, b2)
    key = tuple(dts)
    if key not in _CACHE:
        _CACHE[key] = build_program(dts)
    nc = _CACHE[key]
    res = run_bass_kernel_spmd(nc, in_maps, list(range(NCORES)), trace=False)
    return gather(res.results, h0)
